# revision 1
# baseline (speedup 1.0000x reference)
"""BipartiteGCN message-passing kernel for 8 TRN2 NeuronCores.

Math:  out = D_c^{-1/2} A^T D_r^{-1/2} (x @ W) + b
where A[s, d] = multiplicity of edge (gene s, drug d), s, d in [0, 4000).

Strategy (dst-window sharding, no output all-reduce):
  - Core c owns drug (dst) window [512c, 512c+512).  Edges are sharded to
    cores by dst window and sorted by src gene (host-side layout only; all
    arithmetic happens on device).
  - Each core builds its dense count stripe A_c [4096 genes x 512 drugs]
    directly in SBUF with one-hot x one-hot PE matmuls: for each 128-edge
    chunk, lhsT[e, g] = (src_e == g), rhs[e, d] = (dst_e == d) (fp16
    one-hots built by DVE compare-vs-iota), accumulated per 128-gene window
    in fp32 PSUM.  No gather/scatter DMA at all.
  - xW is computed row-sharded over genes (512 rows/core) and all-gathered.
  - row_deg = free-axis rowsums of A_c (partial -> 16KB AllReduce);
    col_deg = ones^T @ A_c on the PE (local).  f = rsqrt-masked row_deg,
    g = rsqrt-masked col_deg, all on device.
  - out_c = g * ((f*A_c)^T @ xWf) + bias ; host concatenates the stripes.
"""

import sys

if "/opt/trn_rl_repo" not in sys.path:
    sys.path.insert(0, "/opt/trn_rl_repo")

import numpy as np

import concourse.bass as bass  # noqa: F401
import concourse.mybir as mybir
from concourse import bacc, tile

CORES = 8
DWIN = 512              # dst (drug) window per core
ND = 4000               # number of drugs
GD = 4096               # padded gene dim (src < 4000)
IC = 1024
OC = 512
ST = GD // 128          # 32 gene windows / tiles
WCH = 10                # 128-edge chunks per gene window (max 1172 edges)
NCH = ST * WCH          # 320 chunks per core
NSLOT = NCH * 128       # 40960 edge slots per core

F32 = mybir.dt.float32
F16 = mybir.dt.float16


def build_nc(debug_outputs=False):
    nc = bacc.Bacc(
        None,
        target_bir_lowering=False,
        debug=False,
        num_devices=CORES,
    )

    xT = nc.dram_tensor("xT", [IC, DWIN], F32, kind="ExternalInput")
    w = nc.dram_tensor("w", [IC, OC], F32, kind="ExternalInput")
    brep = nc.dram_tensor("brep", [128, OC], F32, kind="ExternalInput")
    i128 = nc.dram_tensor("i128", [128, 128], F16, kind="ExternalInput")
    i512 = nc.dram_tensor("i512", [128, OC], F16, kind="ExternalInput")
    sloc = nc.dram_tensor("sloc", [128, NCH], F32, kind="ExternalInput")
    dloc = nc.dram_tensor("dloc", [128, NCH], F32, kind="ExternalInput")
    out = nc.dram_tensor("out", [DWIN, OC], F32, kind="ExternalOutput")

    xw0l = nc.dram_tensor("xw0l", [DWIN, OC], F32)         # local xW stripe
    xw0f = nc.dram_tensor("xw0f", [GD, OC], F32, addr_space="Shared")
    rdl = nc.dram_tensor("rdl", [128, ST], F32)            # rowdeg partial
    rds = nc.dram_tensor("rds", [128, ST], F32, addr_space="Shared")

    Adbg = xwdbg = None
    if debug_outputs:
        Adbg = nc.dram_tensor("Adbg", [GD, OC], F32, kind="ExternalOutput")
        xwdbg = nc.dram_tensor("xwdbg", [GD, OC], F32, kind="ExternalOutput")

    with tile.TileContext(nc) as tc:
        with (
            tc.tile_pool(name="const", bufs=1) as cpool,
            tc.tile_pool(name="work", bufs=2) as wpool,
            tc.tile_pool(name="apool", bufs=ST) as apool,
            tc.tile_pool(name="psum", bufs=4, space="PSUM") as ppool,
        ):
            # constants
            ones_sb = cpool.tile([128, 1], F32)
            nc.vector.memset(ones_sb[:], 1.0)
            i128_sb = cpool.tile([128, 128], F16)
            nc.sync.dma_start(i128_sb[:], i128[:])
            i512_sb = cpool.tile([128, OC], F16)
            nc.sync.dma_start(i512_sb[:], i512[:])
            bias_sb = cpool.tile([128, OC], F32)
            nc.sync.dma_start(bias_sb[:], brep[:])
            sloc_sb = cpool.tile([128, NCH], F32)
            nc.sync.dma_start(sloc_sb[:], sloc[:])
            dloc_sb = cpool.tile([128, NCH], F32)
            nc.sync.dma_start(dloc_sb[:], dloc[:])

            # phase B: xw0 = x_shard @ W  (genes 512c..512c+512)
            pb = [ppool.tile([128, OC], F32, tag="acc", name=f"pb{i}") for i in range(4)]
            for kt in range(8):
                xt_t = wpool.tile([128, DWIN], F32, tag="xT", name=f"xt{kt}")
                w_t = wpool.tile([128, OC], F32, tag="w", name=f"w{kt}")
                nc.sync.dma_start(xt_t[:], xT[kt * 128:(kt + 1) * 128, :])
                nc.sync.dma_start(w_t[:], w[kt * 128:(kt + 1) * 128, :])
                for mt in range(4):
                    nc.tensor.matmul(
                        pb[mt][:],
                        xt_t[:, mt * 128:(mt + 1) * 128],
                        w_t[:],
                        start=(kt == 0),
                        stop=(kt == 7),
                    )
            for mt in range(4):
                o = wpool.tile([128, OC], F32, tag="xw0sb", name=f"xw0sb{mt}")
                nc.vector.tensor_copy(o[:], pb[mt][:])
                nc.sync.dma_start(xw0l[mt * 128:(mt + 1) * 128, :], o[:])

            # phase C: all-gather xW  (rank r -> rows 512r..512r+512)
            nc.gpsimd.collective_compute(
                "AllGather",
                mybir.AluOpType.bypass,
                replica_groups=[list(range(CORES))],
                ins=[xw0l[:].opt()],
                outs=[xw0f[:].opt()],
            )

            # phase D: build the A stripe in SBUF, one 128-gene window at a
            # time, as sums of one-hot outer products on the PE.  Also emits
            # the row-degree partials (free-axis rowsums).
            a_sb = []
            rd_sb = cpool.tile([128, ST], F32)
            for t in range(ST):
                pa = ppool.tile([128, OC], F32, tag="bld", bufs=2, name=f"pa{t}")
                for i in range(WCH):
                    c = t * WCH + i
                    loh = wpool.tile([128, 128], F16, tag="loh", bufs=3,
                                     name=f"loh{c}")
                    roh = wpool.tile([128, OC], F16, tag="roh", bufs=3,
                                     name=f"roh{c}")
                    nc.vector.tensor_scalar(
                        out=loh[:], in0=i128_sb[:],
                        scalar1=sloc_sb[:, c:c + 1], scalar2=None,
                        op0=mybir.AluOpType.is_equal,
                    )
                    nc.vector.tensor_scalar(
                        out=roh[:], in0=i512_sb[:],
                        scalar1=dloc_sb[:, c:c + 1], scalar2=None,
                        op0=mybir.AluOpType.is_equal,
                    )
                    nc.tensor.matmul(
                        pa[:], loh[:], roh[:],
                        start=(i == 0), stop=(i == WCH - 1),
                    )
                a_t = apool.tile([128, OC], F32, tag="A", name=f"a{t}")
                nc.scalar.copy(a_t[:], pa[:])
                a_sb.append(a_t)
                if debug_outputs:
                    nc.sync.dma_start(Adbg[t * 128:(t + 1) * 128, :], a_t[:])
                nc.vector.reduce_sum(
                    rd_sb[:, t:t + 1], a_t[:], axis=mybir.AxisListType.X
                )

            # col_deg = ones^T @ A  ([1, 512] psum accumulated over windows)
            pcd = ppool.tile([1, OC], F32, tag="cd", bufs=1)
            for t in range(ST):
                nc.tensor.matmul(
                    pcd[:], ones_sb[:], a_sb[t][:],
                    start=(t == 0), stop=(t == ST - 1),
                )
            cd_row = cpool.tile([1, OC], F32)
            nc.vector.tensor_copy(cd_row[:], pcd[:])
            # redistribute [1, 512] -> [128, 4]: column dt holds drugs
            # dt*128 + p on partition p (matches phase G's per-partition g)
            cd_sb = cpool.tile([128, 4], F32)
            for kq in range(4):
                nc.sync.dma_start(
                    cd_sb[:, kq:kq + 1], cd_row[0:1, kq * 128:(kq + 1) * 128]
                )

            # row_deg all-reduce and f = (deg>0)/sqrt(max(deg,1))
            nc.sync.dma_start(rdl[:], rd_sb[:])
            nc.gpsimd.collective_compute(
                "AllReduce",
                mybir.AluOpType.add,
                replica_groups=[list(range(CORES))],
                ins=[rdl[:].opt()],
                outs=[rds[:].opt()],
            )
            deg_sb = cpool.tile([128, ST], F32)
            nc.sync.dma_start(deg_sb[:], rds[:])
            t1 = cpool.tile([128, ST], F32)
            nc.vector.tensor_scalar(
                out=t1[:], in0=deg_sb[:], scalar1=1.0, scalar2=None,
                op0=mybir.AluOpType.max,
            )
            nc.scalar.sqrt(t1[:], t1[:])
            nc.vector.reciprocal(t1[:], t1[:])
            fmask = cpool.tile([128, ST], F32)
            nc.vector.tensor_scalar(
                out=fmask[:], in0=deg_sb[:], scalar1=0.5, scalar2=None,
                op0=mybir.AluOpType.is_gt,
            )
            f_sb = cpool.tile([128, ST], F32)
            nc.vector.tensor_tensor(
                out=f_sb[:], in0=t1[:], in1=fmask[:], op=mybir.AluOpType.mult
            )

            # g = (coldeg>0)/sqrt(max(coldeg,1))   [128, 4]
            g1 = cpool.tile([128, 4], F32)
            nc.vector.tensor_scalar(
                out=g1[:], in0=cd_sb[:], scalar1=1.0, scalar2=None,
                op0=mybir.AluOpType.max,
            )
            nc.scalar.sqrt(g1[:], g1[:])
            nc.vector.reciprocal(g1[:], g1[:])
            gmask = cpool.tile([128, 4], F32)
            nc.vector.tensor_scalar(
                out=gmask[:], in0=cd_sb[:], scalar1=0.5, scalar2=None,
                op0=mybir.AluOpType.is_gt,
            )
            g_sb = cpool.tile([128, 4], F32)
            nc.vector.tensor_tensor(
                out=g_sb[:], in0=g1[:], in1=gmask[:], op=mybir.AluOpType.mult
            )

            # phase F: out = (f*A)^T @ xw0f  accumulated over gene windows
            po = [ppool.tile([128, OC], F32, tag="acc", name=f"po{i}") for i in range(4)]
            for t in range(ST):
                nc.vector.tensor_scalar(
                    out=a_sb[t][:], in0=a_sb[t][:],
                    scalar1=f_sb[:, t:t + 1], scalar2=None,
                    op0=mybir.AluOpType.mult,
                )
                xf_t = wpool.tile([128, OC], F32, tag="xwf", bufs=3, name=f"xf{t}")
                nc.sync.dma_start(xf_t[:], xw0f[t * 128:(t + 1) * 128, :])
                if debug_outputs:
                    nc.sync.dma_start(xwdbg[t * 128:(t + 1) * 128, :], xf_t[:])
                for dt in range(4):
                    nc.tensor.matmul(
                        po[dt][:],
                        a_sb[t][:, dt * 128:(dt + 1) * 128],
                        xf_t[:],
                        start=(t == 0),
                        stop=(t == ST - 1),
                    )

            # phase G: scale by g, add bias, store
            for dt in range(4):
                og = wpool.tile([128, OC], F32, tag="og", name=f"og{dt}")
                nc.vector.tensor_scalar(
                    out=og[:], in0=po[dt][:],
                    scalar1=g_sb[:, dt:dt + 1], scalar2=None,
                    op0=mybir.AluOpType.mult,
                )
                nc.vector.tensor_tensor(
                    out=og[:], in0=og[:], in1=bias_sb[:], op=mybir.AluOpType.add
                )
                nc.sync.dma_start(out[dt * 128:(dt + 1) * 128, :], og[:])

    nc.finalize()
    return nc


def make_in_maps(x, weight, bias, edge_index):
    """Host-side sharding/layout only: no arithmetic on tensor values."""
    x = np.asarray(x, dtype=np.float32)
    weight = np.ascontiguousarray(np.asarray(weight, dtype=np.float32))
    bias = np.asarray(bias, dtype=np.float32)
    ei = np.asarray(edge_index)
    s_all = ei[0].astype(np.int64)
    d_all = ei[1].astype(np.int64)
    assert s_all.min() >= 0 and s_all.max() < ND, "src ids out of supported range"
    assert d_all.min() >= 0 and d_all.max() < ND, "dst ids out of supported range"

    brep = np.ascontiguousarray(np.tile(bias[None, :], (128, 1)).astype(np.float32))
    i128 = np.ascontiguousarray(
        np.tile(np.arange(128, dtype=np.float16)[None, :], (128, 1))
    )
    i512 = np.ascontiguousarray(
        np.tile(np.arange(OC, dtype=np.float16)[None, :], (128, 1))
    )

    core_of = d_all >> 9
    in_maps = []
    for c in range(CORES):
        m = core_of == c
        s = s_all[m]
        dl = d_all[m] - c * DWIN

        # window-major slot packing: gene window w = s >> 7 gets WCH chunks
        # of 128 slots; pads get -1 (all-zero one-hots)
        sl_lin = np.full(NSLOT, -1.0, dtype=np.float32)
        dl_lin = np.full(NSLOT, -1.0, dtype=np.float32)
        o = np.argsort(s, kind="stable")
        s_o = s[o]
        dl_o = dl[o]
        wnd = s_o >> 7
        cnt = np.bincount(wnd, minlength=ST)
        assert cnt.max() <= WCH * 128, f"window overflow: {cnt.max()}"
        pos = 0
        for t in range(ST):
            n = int(cnt[t])
            base = t * WCH * 128
            sl_lin[base:base + n] = (s_o[pos:pos + n] - t * 128).astype(np.float32)
            dl_lin[base:base + n] = dl_o[pos:pos + n].astype(np.float32)
            pos += n

        sloc_t = np.ascontiguousarray(sl_lin.reshape(NCH, 128).T)
        dloc_t = np.ascontiguousarray(dl_lin.reshape(NCH, 128).T)

        xsT = np.ascontiguousarray(x[c * DWIN:(c + 1) * DWIN, :].T)

        in_maps.append(
            {
                "xT": xsT,
                "w": weight,
                "brep": brep,
                "i128": i128,
                "i512": i512,
                "sloc": sloc_t,
                "dloc": dloc_t,
            }
        )
    return in_maps


_NC = None


def _get_nc():
    global _NC
    if _NC is None:
        _NC = build_nc()
    return _NC


def kernel(x, weight, bias, edge_index, **run_kwargs):
    from concourse.bass_utils import run_bass_kernel_spmd

    nc = _get_nc()
    in_maps = make_in_maps(x, weight, bias, edge_index)
    res = run_bass_kernel_spmd(nc, in_maps, core_ids=list(range(CORES)), **run_kwargs)
    outs = res.results if hasattr(res, "results") else res
    full = np.empty((ND, OC), dtype=np.float32)
    for c in range(CORES):
        n = min(DWIN, ND - c * DWIN)
        full[c * DWIN:c * DWIN + n] = outs[c]["out"][:n]
    if run_kwargs:
        return full, res
    return full



# revision 2
# speedup vs baseline: 3.6479x; 3.6479x over previous
"""BipartiteGCN message-passing kernel for 8 TRN2 NeuronCores.

Math:  out = D_c^{-1/2} A^T D_r^{-1/2} (x @ W) + b
where A[s, d] = multiplicity of edge (gene s, drug d), s, d in [0, 4000).

Strategy (dst-window sharding, no collectives):
  - Core c owns drug (dst) window [512c, 512c+512).  Host groups that
    core's edges by gene row and 128-gene window, dedupes per (gene,dst)
    pair into (dst-index, multiplicity) slots (pure index layout).
  - On device, each 128-gene window of the dense count stripe
    A_w [128 x 512] (f16) is built in SBUF by ONE gpsimd local_scatter
    (per-partition indices, multiplicity payload).  No one-hot matmuls.
  - row_deg is exact and collective-free: every core gets all cores'
    multiplicity slot tables (layout replication) and row-reduces them.
    f = rsqrt-masked row_deg scales A in place (DVE/Act alternating).
  - col_deg comes free from the main GEMM operand: 1-wide matmuls
    (f*A)^T @ (1/f) accumulate col sums in PSUM.
  - xw = x @ W computed in f32r (1 cycle/row), output cast to f16.
  - out = g * ((f*A)^T @ xw) + bias ; host concatenates dst stripes.
"""

import sys

if "/opt/trn_rl_repo" not in sys.path:
    sys.path.insert(0, "/opt/trn_rl_repo")

import numpy as np

import concourse.bass as bass  # noqa: F401
import concourse.mybir as mybir
from concourse import bacc, tile

CORES = 8
DWIN = 512              # dst (drug) window per core
ND = 4000               # number of drugs
GD = 4096               # padded gene dim (src < 4000)
NW = GD // 128          # 32 gene windows
IC = 1024
OC = 512
KMAX = 32               # max deduped (gene,dst) slots per gene row per window
GTPL = 4                # gene tiles per x-load DMA

F32 = mybir.dt.float32
F32R = mybir.dt.float32r
F16 = mybir.dt.float16
I16 = mybir.dt.int16


def build_nc(debug_outputs=False):
    nc = bacc.Bacc(
        None,
        target_bir_lowering=False,
        debug=False,
        num_devices=CORES,
    )

    # x^T rearranged on host: xh[p, k, c] = x[c, k*128+p] (layout only)
    xh = nc.dram_tensor("xh", [128, IC // 128, GD], F32R, kind="ExternalInput")
    wh = nc.dram_tensor("wh", [128, IC // 128, OC], F32R, kind="ExternalInput")
    brep = nc.dram_tensor("brep", [128, OC], F32, kind="ExternalInput")
    idxt = nc.dram_tensor("idxt", [128, NW * KMAX], I16, kind="ExternalInput")
    datt = nc.dram_tensor("datt", [128, NW * KMAX], F16, kind="ExternalInput")
    mall = nc.dram_tensor("mall", [128, NW, CORES * KMAX], F16,
                          kind="ExternalInput")
    out = nc.dram_tensor("out", [DWIN, OC], F32, kind="ExternalOutput")

    Adbg = None
    if debug_outputs:
        Adbg = nc.dram_tensor("Adbg", [GD, OC], F32, kind="ExternalOutput")

    with tile.TileContext(nc) as tc:
        with (
            tc.tile_pool(name="const", bufs=1) as cpool,
            tc.tile_pool(name="work", bufs=2) as wpool,
            tc.tile_pool(name="apool", bufs=NW) as apool,
            tc.tile_pool(name="xwpool", bufs=NW) as xwpool,
            tc.tile_pool(name="psum", bufs=4, space="PSUM") as ppool,
        ):
            # --- constant / input loads -------------------------------
            idx_sb = cpool.tile([128, NW * KMAX], I16)
            nc.sync.dma_start(idx_sb[:], idxt[:])
            dat_sb = cpool.tile([128, NW * KMAX], F16)
            nc.sync.dma_start(dat_sb[:], datt[:])
            mall_sb = cpool.tile([128, NW, CORES * KMAX], F16)
            nc.sync.dma_start(mall_sb[:], mall[:])
            brep_sb = cpool.tile([128, OC], F32)
            nc.sync.dma_start(brep_sb[:], brep[:])
            w_sb = cpool.tile([128, IC // 128, OC], F32R)
            nc.sync.dma_start(w_sb[:], wh[:])

            # --- A stripe: one local_scatter per gene window ----------
            a_sb = []
            for w in range(NW):
                a_t = apool.tile([128, OC], F16, tag="A", name=f"a{w}")
                nc.gpsimd.local_scatter(
                    out_ap=a_t[:],
                    data_ap=dat_sb[:, w * KMAX:(w + 1) * KMAX],
                    idxs_ap=idx_sb[:, w * KMAX:(w + 1) * KMAX],
                    channels=128,
                    num_elems=OC,
                    num_idxs=KMAX,
                )
                a_sb.append(a_t)

            # --- row_deg from the replicated multiplicity tables ------
            rd16 = cpool.tile([128, NW], F16)
            with nc.allow_low_precision("integer-valued degree sums <2048 are exact in f16"):
                nc.vector.reduce_sum(rd16[:], mall_sb[:], axis=mybir.AxisListType.X)
            rd = cpool.tile([128, NW], F32)
            nc.vector.tensor_copy(rd[:], rd16[:])

            # f = (deg>0)/sqrt(max(deg,1)); finv = (deg>0)*sqrt(max(deg,1))
            t1 = cpool.tile([128, NW], F32)
            nc.vector.tensor_scalar(
                out=t1[:], in0=rd[:], scalar1=1.0, scalar2=None,
                op0=mybir.AluOpType.max,
            )
            nc.scalar.sqrt(t1[:], t1[:])
            rcp = cpool.tile([128, NW], F32)
            nc.vector.reciprocal(rcp[:], t1[:])
            fmask = cpool.tile([128, NW], F32)
            nc.vector.tensor_scalar(
                out=fmask[:], in0=rd[:], scalar1=0.5, scalar2=None,
                op0=mybir.AluOpType.is_gt,
            )
            f_sb = cpool.tile([128, NW], F32)
            nc.vector.tensor_tensor(
                out=f_sb[:], in0=rcp[:], in1=fmask[:], op=mybir.AluOpType.mult
            )
            finv16 = cpool.tile([128, NW], F16)
            nc.vector.tensor_tensor(
                out=finv16[:], in0=t1[:], in1=fmask[:], op=mybir.AluOpType.mult
            )

            # --- xw = x @ W in f32r, cast to f16 ----------------------
            xw_sb = []
            for gb in range(NW // GTPL):
                xg = wpool.tile([128, IC // 128, GTPL * 128], F32R,
                                tag="xg", bufs=2, name=f"xg{gb}")
                nc.sync.dma_start(
                    xg[:], xh[:, :, gb * GTPL * 128:(gb + 1) * GTPL * 128]
                )
                for gs in range(GTPL):
                    g = gb * GTPL + gs
                    pxw = ppool.tile([128, OC], F32, tag="xwp", bufs=2,
                                     name=f"pxw{g}")
                    for k in range(IC // 128):
                        nc.tensor.matmul(
                            pxw[:],
                            xg[:, k, gs * 128:(gs + 1) * 128],
                            w_sb[:, k, :],
                            start=(k == 0),
                            stop=(k == IC // 128 - 1),
                        )
                    xw_t = xwpool.tile([128, OC], F16, tag="XW", name=f"xw{g}")
                    nc.scalar.copy(xw_t[:], pxw[:])
                    xw_sb.append(xw_t)

            # --- scale A rows by f (in place), alternating DVE/Act ----
            for w in range(NW):
                if w % 2 == 0:
                    nc.vector.tensor_scalar(
                        out=a_sb[w][:], in0=a_sb[w][:],
                        scalar1=f_sb[:, w:w + 1], scalar2=None,
                        op0=mybir.AluOpType.mult,
                    )
                else:
                    nc.scalar.mul(a_sb[w][:], a_sb[w][:], f_sb[:, w:w + 1])
                if debug_outputs:
                    dbg = wpool.tile([128, OC], F32, tag="dbg", name=f"dbg{w}")
                    nc.vector.tensor_copy(dbg[:], a_sb[w][:])
                    nc.sync.dma_start(Adbg[w * 128:(w + 1) * 128, :], dbg[:])

            # --- main GEMM + colsum accumulation ----------------------
            po = [ppool.tile([128, OC], F32, tag="acc", name=f"po{q}")
                  for q in range(4)]
            pcd = ppool.tile([128, 4], F32, tag="cd", bufs=1)
            nc.vector.memset(pcd[:], 0.0)
            for w in range(NW):
                for q in range(4):
                    nc.tensor.matmul(
                        po[q][:],
                        a_sb[w][:, q * 128:(q + 1) * 128],
                        xw_sb[w][:],
                        start=(w == 0),
                        stop=(w == NW - 1),
                    )
                    nc.tensor.matmul(
                        pcd[:, q:q + 1],
                        a_sb[w][:, q * 128:(q + 1) * 128],
                        finv16[:, w:w + 1],
                        start=False,
                        stop=False,
                        skip_group_check=True,
                    )

            # --- g = (coldeg>0)/sqrt(max(coldeg,1)) -------------------
            cd = cpool.tile([128, 4], F32)
            nc.vector.tensor_scalar(
                out=cd[:], in0=pcd[:], scalar1=1.0, scalar2=None,
                op0=mybir.AluOpType.max,
            )
            nc.scalar.sqrt(cd[:], cd[:])
            grcp = cpool.tile([128, 4], F32)
            nc.vector.reciprocal(grcp[:], cd[:])
            gmask = cpool.tile([128, 4], F32)
            nc.vector.tensor_scalar(
                out=gmask[:], in0=pcd[:], scalar1=0.5, scalar2=None,
                op0=mybir.AluOpType.is_gt,
            )
            g_sb = cpool.tile([128, 4], F32)
            nc.vector.tensor_tensor(
                out=g_sb[:], in0=grcp[:], in1=gmask[:], op=mybir.AluOpType.mult
            )

            # --- out = g * po + bias ----------------------------------
            for q in range(4):
                og = wpool.tile([128, OC], F32, tag="og", name=f"og{q}")
                nc.scalar.mul(og[:], po[q][:], g_sb[:, q:q + 1])
                nc.vector.tensor_tensor(
                    out=og[:], in0=og[:], in1=brep_sb[:], op=mybir.AluOpType.add
                )
                nc.sync.dma_start(out[q * 128:(q + 1) * 128, :], og[:])

    nc.finalize()
    return nc


def make_in_maps(x, weight, bias, edge_index):
    """Host-side sharding/layout only: grouping, dedup and padding of the
    edge list (index preprocessing); no arithmetic on float tensor data."""
    x = np.asarray(x, dtype=np.float32)
    weight = np.ascontiguousarray(np.asarray(weight, dtype=np.float32))
    bias = np.asarray(bias, dtype=np.float32)
    ei = np.asarray(edge_index)
    s_all = ei[0].astype(np.int64)
    d_all = ei[1].astype(np.int64)
    assert s_all.min() >= 0 and s_all.max() < ND, "src ids out of supported range"
    assert d_all.min() >= 0 and d_all.max() < ND, "dst ids out of supported range"

    brep = np.ascontiguousarray(np.tile(bias[None, :], (128, 1)).astype(np.float32))

    # xh[p, k, c] = x[c, k*128 + p]  (transpose layout for matmul lhsT)
    xT = np.zeros((IC, GD), dtype=np.float32)
    xT[:, :ND] = x[:ND].T
    xh = np.ascontiguousarray(xT.reshape(IC // 128, 128, GD).transpose(1, 0, 2))
    whr = np.ascontiguousarray(weight.reshape(IC // 128, 128, OC).transpose(1, 0, 2))

    core_of = d_all >> 9
    idxs = []
    dats = []
    for c in range(CORES):
        m = core_of == c
        key = s_all[m] * DWIN + (d_all[m] - c * DWIN)
        uniq, cnt = np.unique(key, return_counts=True)
        g = uniq // DWIN
        dloc = uniq % DWIN
        w = g >> 7
        p = g & 127
        # slot rank within each (w, p) group (uniq is sorted so g is sorted)
        gk = w * 128 + p
        _, start_idx = np.unique(gk, return_index=True)
        starts = np.zeros(len(gk), dtype=np.int64)
        starts[start_idx] = start_idx
        starts = np.maximum.accumulate(starts)
        rank = np.arange(len(gk)) - starts
        assert rank.max() < KMAX, f"slot overflow: {rank.max() + 1} > {KMAX}"

        idx_t = np.full((128, NW * KMAX), -1, dtype=np.int16)
        dat_t = np.zeros((128, NW * KMAX), dtype=np.float16)
        col = w * KMAX + rank
        idx_t[p, col] = dloc.astype(np.int16)
        dat_t[p, col] = cnt.astype(np.float16)
        idxs.append(idx_t)
        dats.append(dat_t)

    # mall[p, w, c*KMAX + k] = dats[c][p, w*KMAX + k]
    mall = np.zeros((128, NW, CORES * KMAX), dtype=np.float16)
    for c in range(CORES):
        mall[:, :, c * KMAX:(c + 1) * KMAX] = dats[c].reshape(128, NW, KMAX)
    mall = np.ascontiguousarray(mall)

    in_maps = []
    for c in range(CORES):
        in_maps.append(
            {
                "xh": xh,
                "wh": whr,
                "brep": brep,
                "idxt": idxs[c],
                "datt": dats[c],
                "mall": mall,
            }
        )
    return in_maps


_NC = None


def _get_nc():
    global _NC
    if _NC is None:
        _NC = build_nc()
    return _NC


def kernel(x, weight, bias, edge_index, **run_kwargs):
    from concourse.bass_utils import run_bass_kernel_spmd

    nc = _get_nc()
    in_maps = make_in_maps(x, weight, bias, edge_index)
    res = run_bass_kernel_spmd(nc, in_maps, core_ids=list(range(CORES)), **run_kwargs)
    outs = res.results if hasattr(res, "results") else res
    full = np.empty((ND, OC), dtype=np.float32)
    for c in range(CORES):
        n = min(DWIN, ND - c * DWIN)
        full[c * DWIN:c * DWIN + n] = outs[c]["out"][:n]
    if run_kwargs:
        return full, res
    return full


# revision 5
# speedup vs baseline: 4.3087x; 1.1811x over previous
"""BipartiteGCN message-passing kernel for 8 TRN2 NeuronCores.

Math:  out = D_c^{-1/2} A^T D_r^{-1/2} (x @ W) + b
where A[s, d] = multiplicity of edge (gene s, drug d), s, d in [0, 4000).

Strategy (gene sharding + one f16 ReduceScatter):
  - Core c owns gene range [512c, 512c+512) and ALL drug columns.  The
    host groups that core's edges by (gene row, 128-gene window, dst
    quarter), dedupes per (gene,dst) pair into (dst-index, multiplicity)
    slots (pure index layout, no float arithmetic).
  - Each [128 x 1024] block of the dense count stripe A (f16) is built
    in SBUF by ONE gpsimd local_scatter (per-partition indices,
    multiplicity payload).  No one-hot matmuls, no gather/scatter DMA.
  - row_deg is fully local (a core owns complete gene rows): one
    f16 reduce over the multiplicity table.  f = rsqrt-masked row_deg
    scales A in place.  No AllReduce.
  - col_deg rides along as payload column 512 of the partial GEMM
    output: tiny rank-1 matmuls (f*A)^T @ (1/f) accumulate per-dst
    column sums in PSUM.
  - xw = x_shard @ W in f32r (1 cycle/row), cast to f16.
  - partials[d, :512] = (f*A)^T @ xw over the 512 local genes; one
    ReduceScatter(add) over the [4096, 520] f16 partials both sums the
    8 per-core partials and hands core c exactly its 512-drug window.
  - out = g * partial + bias ; host concatenates the dst stripes.
"""

import sys

if "/opt/trn_rl_repo" not in sys.path:
    sys.path.insert(0, "/opt/trn_rl_repo")

import numpy as np

import concourse.bass as bass  # noqa: F401
import concourse.mybir as mybir
from concourse import bacc, tile

CORES = 8
GSH = 512               # genes per core
NWJ = GSH // 128        # 4 local gene windows
ND = 4000               # number of drugs
NDP = 4096              # padded drug dim
NQ = 4                  # dst quarters
QW = NDP // NQ          # 1024 dst per quarter
IC = 1024
OC = 512
PC = 520                # RS payload columns: 512 out + col_deg + pad
KMAX = 48               # max deduped slots per (gene row, window, quarter)

F32 = mybir.dt.float32
F32R = mybir.dt.float32r
F16 = mybir.dt.float16
I16 = mybir.dt.int16


def build_nc(debug_outputs=False):
    nc = bacc.Bacc(
        None,
        target_bir_lowering=False,
        debug=False,
        num_devices=CORES,
    )

    # xh[p, k, gl] = x[512c + gl, 128k + p]  (lhsT layout for x @ W)
    xh = nc.dram_tensor("xh", [128, IC // 128, GSH], F32R, kind="ExternalInput")
    wh = nc.dram_tensor("wh", [128, IC // 128, OC], F32R, kind="ExternalInput")
    brep = nc.dram_tensor("brep", [128, OC], F32, kind="ExternalInput")
    # slot tables: col = j*(NQ*KMAX) + q*KMAX + k
    idxt = nc.dram_tensor("idxt", [128, NWJ * NQ * KMAX], I16, kind="ExternalInput")
    datt = nc.dram_tensor("datt", [128, NWJ, NQ * KMAX], F16, kind="ExternalInput")
    out = nc.dram_tensor("out", [GSH, OC], F32, kind="ExternalOutput")

    rsin = nc.dram_tensor("rsin", [NDP, PC], F16)
    rsout = nc.dram_tensor("rsout", [GSH, PC], F16)

    with tile.TileContext(nc) as tc:
        with (
            tc.tile_pool(name="const", bufs=1) as cpool,
            tc.tile_pool(name="work", bufs=2) as wpool,
            tc.tile_pool(name="apool", bufs=NWJ * NQ) as apool,
            tc.tile_pool(name="psum", bufs=4, space="PSUM") as ppool,
        ):
            # --- input loads ------------------------------------------
            idx_sb = cpool.tile([128, NWJ * NQ * KMAX], I16)
            nc.sync.dma_start(idx_sb[:], idxt[:])
            dat_sb = cpool.tile([128, NWJ, NQ * KMAX], F16)
            nc.sync.dma_start(dat_sb[:], datt[:])
            w_sb = cpool.tile([128, IC // 128, OC], F32R)
            nc.sync.dma_start(w_sb[:], wh[:])
            x_sb = cpool.tile([128, IC // 128, GSH], F32R)
            nc.sync.dma_start(x_sb[:], xh[:])
            brep_sb = cpool.tile([128, OC], F32)
            nc.sync.dma_start(brep_sb[:], brep[:])

            # --- row_deg (local!) + f, 1/f ----------------------------
            rd16 = cpool.tile([128, NWJ], F16)
            with nc.allow_low_precision("int-valued degree sums <2048 exact in f16"):
                nc.vector.reduce_sum(rd16[:], dat_sb[:], axis=mybir.AxisListType.X)
            rd = cpool.tile([128, NWJ], F32)
            nc.vector.tensor_copy(rd[:], rd16[:])
            t1 = cpool.tile([128, NWJ], F32)
            nc.vector.tensor_scalar(
                out=t1[:], in0=rd[:], scalar1=1.0, scalar2=None,
                op0=mybir.AluOpType.max,
            )
            nc.scalar.sqrt(t1[:], t1[:])
            rcp = cpool.tile([128, NWJ], F32)
            nc.vector.reciprocal(rcp[:], t1[:])
            fmask = cpool.tile([128, NWJ], F32)
            nc.vector.tensor_scalar(
                out=fmask[:], in0=rd[:], scalar1=0.5, scalar2=None,
                op0=mybir.AluOpType.is_gt,
            )
            f_sb = cpool.tile([128, NWJ], F32)
            nc.vector.tensor_tensor(
                out=f_sb[:], in0=rcp[:], in1=fmask[:], op=mybir.AluOpType.mult
            )
            finv16 = cpool.tile([128, NWJ], F16)
            nc.vector.tensor_tensor(
                out=finv16[:], in0=t1[:], in1=fmask[:], op=mybir.AluOpType.mult
            )

            # --- A blocks: one local_scatter per (quarter, window) ----
            a_sb = {}
            for q in range(NQ):
                for j in range(NWJ):
                    a_t = apool.tile([128, QW], F16, tag="A", name=f"a{q}_{j}")
                    base = j * (NQ * KMAX) + q * KMAX
                    nc.gpsimd.local_scatter(
                        out_ap=a_t[:],
                        data_ap=dat_sb[:, j, q * KMAX:(q + 1) * KMAX],
                        idxs_ap=idx_sb[:, base:base + KMAX],
                        channels=128,
                        num_elems=QW,
                        num_idxs=KMAX,
                    )
                    a_sb[(q, j)] = a_t

            # --- xw = x_shard @ W (f32r), cast to f16 -----------------
            xw_sb = []
            for j in range(NWJ):
                pxw = ppool.tile([128, OC], F32, tag="xwp", bufs=2, name=f"pxw{j}")
                for k in range(IC // 128):
                    nc.tensor.matmul(
                        pxw[:],
                        x_sb[:, k, j * 128:(j + 1) * 128],
                        w_sb[:, k, :],
                        start=(k == 0),
                        stop=(k == IC // 128 - 1),
                    )
                xw_t = cpool.tile([128, OC], F16, tag="XW", bufs=NWJ,
                                  name=f"xw{j}")
                nc.scalar.copy(xw_t[:], pxw[:])
                xw_sb.append(xw_t)

            # --- scale A rows by f in place (DVE/Act alternating) -----
            for q in range(NQ):
                for j in range(NWJ):
                    if j % 2 == 0:
                        nc.vector.tensor_scalar(
                            out=a_sb[(q, j)][:], in0=a_sb[(q, j)][:],
                            scalar1=f_sb[:, j:j + 1], scalar2=None,
                            op0=mybir.AluOpType.mult,
                        )
                    else:
                        nc.scalar.mul(
                            a_sb[(q, j)][:], a_sb[(q, j)][:], f_sb[:, j:j + 1]
                        )

            # --- partial GEMM + col_deg payload, flush to rsin --------
            for q in range(NQ):
                for t in range(QW // 128):
                    pt = q * (QW // 128) + t
                    pp = ppool.tile([128, OC], F32, tag="pp", bufs=2,
                                    name=f"pp{pt}")
                    pcd = ppool.tile([128, OC], F32, tag="cd", bufs=2,
                                     name=f"pcd{pt}")
                    for j in range(NWJ):
                        nc.tensor.matmul(
                            pp[:],
                            a_sb[(q, j)][:, t * 128:(t + 1) * 128],
                            xw_sb[j][:],
                            start=(j == 0),
                            stop=(j == NWJ - 1),
                        )
                        nc.tensor.matmul(
                            pcd[:, 0:1],
                            a_sb[(q, j)][:, t * 128:(t + 1) * 128],
                            finv16[:, j:j + 1],
                            start=(j == 0),
                            stop=(j == NWJ - 1),
                        )
                    st = wpool.tile([128, PC], F16, tag="st", bufs=3,
                                    name=f"st{pt}")
                    if pt % 2 == 0:
                        nc.scalar.copy(st[:, 0:OC], pp[:])
                    else:
                        nc.vector.tensor_copy(st[:, 0:OC], pp[:])
                    nc.vector.tensor_copy(st[:, OC:OC + 1], pcd[:, 0:1])
                    nc.vector.memset(st[:, OC + 1:PC], 0.0)
                    nc.sync.dma_start(rsin[pt * 128:(pt + 1) * 128, :], st[:])

            # --- ReduceScatter: sum partials, keep my dst window ------
            nc.gpsimd.collective_compute(
                "ReduceScatter",
                mybir.AluOpType.add,
                replica_groups=[list(range(CORES))],
                ins=[rsin[:].opt()],
                outs=[rsout[:].opt()],
            )

            # --- finalize: g scale + bias -----------------------------
            rs_sb = cpool.tile([128, NWJ, PC], F16)
            nc.sync.dma_start(rs_sb[:], rsout[:])
            cdc = cpool.tile([128, NWJ], F32)
            nc.vector.tensor_copy(cdc[:], rs_sb[:, :, OC])
            g1 = cpool.tile([128, NWJ], F32)
            nc.vector.tensor_scalar(
                out=g1[:], in0=cdc[:], scalar1=1.0, scalar2=None,
                op0=mybir.AluOpType.max,
            )
            nc.scalar.sqrt(g1[:], g1[:])
            grcp = cpool.tile([128, NWJ], F32)
            nc.vector.reciprocal(grcp[:], g1[:])
            gmask = cpool.tile([128, NWJ], F32)
            nc.vector.tensor_scalar(
                out=gmask[:], in0=cdc[:], scalar1=0.5, scalar2=None,
                op0=mybir.AluOpType.is_gt,
            )
            g_sb = cpool.tile([128, NWJ], F32)
            nc.vector.tensor_tensor(
                out=g_sb[:], in0=grcp[:], in1=gmask[:], op=mybir.AluOpType.mult
            )
            for v in range(NWJ):
                og = wpool.tile([128, OC], F32, tag="og", name=f"og{v}")
                nc.scalar.mul(og[:], rs_sb[:, v, 0:OC], g_sb[:, v:v + 1])
                nc.vector.tensor_tensor(
                    out=og[:], in0=og[:], in1=brep_sb[:], op=mybir.AluOpType.add
                )
                nc.sync.dma_start(out[v * 128:(v + 1) * 128, :], og[:])

    nc.finalize()
    return nc


def make_in_maps(x, weight, bias, edge_index):
    """Host-side sharding/layout only: grouping, dedup and padding of the
    edge list (index preprocessing); no arithmetic on float tensor data."""
    x = np.asarray(x, dtype=np.float32)
    weight = np.ascontiguousarray(np.asarray(weight, dtype=np.float32))
    bias = np.asarray(bias, dtype=np.float32)
    ei = np.asarray(edge_index)
    s_all = ei[0].astype(np.int64)
    d_all = ei[1].astype(np.int64)
    assert s_all.min() >= 0 and s_all.max() < ND, "src ids out of supported range"
    assert d_all.min() >= 0 and d_all.max() < ND, "dst ids out of supported range"

    brep = np.ascontiguousarray(np.tile(bias[None, :], (128, 1)).astype(np.float32))
    whr = np.ascontiguousarray(weight.reshape(IC // 128, 128, OC).transpose(1, 0, 2))

    core_of = s_all >> 9
    in_maps = []
    for c in range(CORES):
        m = core_of == c
        gl = s_all[m] - c * GSH           # local gene id [0, 512)
        d = d_all[m]                      # full dst id [0, 4000)
        key = gl * NDP + d
        uniq, cnt = np.unique(key, return_counts=True)
        gu = uniq // NDP
        du = uniq % NDP
        j = gu >> 7
        p = gu & 127
        q = du >> 10
        dloc = du & (QW - 1)
        # slot rank within each (j, q, p) group (uniq sorted => groups together)
        gk = (gu << 2) | q
        _, start_idx = np.unique(gk, return_index=True)
        starts = np.zeros(len(gk), dtype=np.int64)
        starts[start_idx] = start_idx
        starts = np.maximum.accumulate(starts)
        rank = np.arange(len(gk)) - starts
        assert rank.max() < KMAX, f"slot overflow: {rank.max() + 1} > {KMAX}"

        idx_t = np.full((128, NWJ * NQ * KMAX), -1, dtype=np.int16)
        dat_t = np.zeros((128, NWJ * NQ * KMAX), dtype=np.float16)
        col = j * (NQ * KMAX) + q * KMAX + rank
        idx_t[p, col] = dloc.astype(np.int16)
        dat_t[p, col] = cnt.astype(np.float16)

        xs = np.zeros((GSH, IC), dtype=np.float32)
        n = min(GSH, ND - c * GSH)
        xs[:n] = x[c * GSH:c * GSH + n]
        xhr = np.ascontiguousarray(xs.T.reshape(IC // 128, 128, GSH).transpose(1, 0, 2))

        in_maps.append(
            {
                "xh": xhr,
                "wh": whr,
                "brep": brep,
                "idxt": idx_t,
                "datt": dat_t.reshape(128, NWJ, NQ * KMAX),
            }
        )
    return in_maps


_NC = None


def _get_nc():
    global _NC
    if _NC is None:
        _NC = build_nc()
    return _NC


def kernel(x, weight, bias, edge_index, **run_kwargs):
    from concourse.bass_utils import run_bass_kernel_spmd

    nc = _get_nc()
    in_maps = make_in_maps(x, weight, bias, edge_index)
    res = run_bass_kernel_spmd(nc, in_maps, core_ids=list(range(CORES)), **run_kwargs)
    outs = res.results if hasattr(res, "results") else res
    full = np.empty((ND, OC), dtype=np.float32)
    for c in range(CORES):
        n = min(GSH, ND - c * GSH)
        full[c * GSH:c * GSH + n] = outs[c]["out"][:n]
    if run_kwargs:
        return full, res
    return full


# revision 7
# speedup vs baseline: 4.3356x; 1.0062x over previous
"""BipartiteGCN message-passing kernel for 8 TRN2 NeuronCores.

Math:  out = D_c^{-1/2} A^T D_r^{-1/2} (x @ W) + b
where A[s, d] = multiplicity of edge (gene s, drug d), s, d in [0, 4000).

Strategy (gene sharding + one f16 ReduceScatter):
  - Core c owns gene range [512c, 512c+512) and ALL drug columns.  The
    host groups that core's edges by (gene row, 128-gene window, dst
    quarter), dedupes per (gene,dst) pair into (dst-index, multiplicity)
    slots (pure index layout, no float arithmetic).
  - Each [128 x 1024] block of the dense count stripe A (f16) is built
    in SBUF by ONE gpsimd local_scatter (per-partition indices,
    multiplicity payload).  No one-hot matmuls, no gather/scatter DMA.
  - row_deg is fully local (a core owns complete gene rows): one
    f16 reduce over the multiplicity table.  f = rsqrt-masked row_deg
    scales A in place.  No AllReduce.
  - col_deg rides along as payload column 512 of the partial GEMM
    output: tiny rank-1 matmuls (f*A)^T @ (1/f) accumulate per-dst
    column sums in PSUM.
  - xw = x_shard @ W in f32r (1 cycle/row), cast to f16.
  - partials[d, :512] = (f*A)^T @ xw over the 512 local genes; one
    ReduceScatter(add) over the [4096, 520] f16 partials both sums the
    8 per-core partials and hands core c exactly its 512-drug window.
  - out = g * partial + bias ; host concatenates the dst stripes.
"""

import sys

if "/opt/trn_rl_repo" not in sys.path:
    sys.path.insert(0, "/opt/trn_rl_repo")

import numpy as np

import concourse.bass as bass  # noqa: F401
import concourse.mybir as mybir
from concourse import bacc, tile

CORES = 8
GSH = 512               # genes per core
NWJ = GSH // 128        # 4 local gene windows
ND = 4000               # number of drugs
NDP = 4096              # padded drug dim
NQ = 4                  # dst quarters
QW = NDP // NQ          # 1024 dst per quarter
IC = 1024
OC = 512
PC = 520                # RS payload columns: 512 out + col_deg + pad
KMAX = 48               # max deduped slots per (gene row, window, quarter)

F32 = mybir.dt.float32
F32R = mybir.dt.float32r
F16 = mybir.dt.float16
I16 = mybir.dt.int16


def build_nc(debug_outputs=False):
    nc = bacc.Bacc(
        None,
        target_bir_lowering=False,
        debug=False,
        num_devices=CORES,
    )

    # xh[p, k, gl] = x[512c + gl, 128k + p]  (lhsT layout for x @ W)
    xh = nc.dram_tensor("xh", [128, IC // 128, GSH], F32R, kind="ExternalInput")
    wh = nc.dram_tensor("wh", [128, IC // 128, OC], F32R, kind="ExternalInput")
    brep = nc.dram_tensor("brep", [128, OC], F32, kind="ExternalInput")
    # slot tables: col = j*(NQ*KMAX) + q*KMAX + k
    idxt = nc.dram_tensor("idxt", [128, NWJ * NQ * KMAX], I16, kind="ExternalInput")
    datt = nc.dram_tensor("datt", [128, NWJ, NQ * KMAX], F16, kind="ExternalInput")
    out = nc.dram_tensor("out", [GSH, OC], F32, kind="ExternalOutput")

    rsin = nc.dram_tensor("rsin", [NDP, PC], F16)
    rsout = nc.dram_tensor("rsout", [NWJ, 128, PC], F16)

    with tile.TileContext(nc) as tc:
        with (
            tc.tile_pool(name="const", bufs=1) as cpool,
            tc.tile_pool(name="work", bufs=2) as wpool,
            tc.tile_pool(name="apool", bufs=NWJ * NQ) as apool,
            tc.tile_pool(name="psum", bufs=4, space="PSUM") as ppool,
        ):
            # --- input loads ------------------------------------------
            idx_sb = cpool.tile([128, NWJ * NQ * KMAX], I16)
            nc.sync.dma_start(idx_sb[:], idxt[:])
            dat_sb = cpool.tile([128, NWJ, NQ * KMAX], F16)
            nc.sync.dma_start(dat_sb[:], datt[:])
            w_sb = cpool.tile([128, IC // 128, OC], F32R)
            nc.sync.dma_start(w_sb[:], wh[:])
            x_sb = cpool.tile([128, IC // 128, GSH], F32R)
            nc.sync.dma_start(x_sb[:], xh[:])
            brep_sb = cpool.tile([128, OC], F32)
            nc.sync.dma_start(brep_sb[:], brep[:])

            # --- row_deg (local!) + f, 1/f ----------------------------
            rd16 = cpool.tile([128, NWJ], F16)
            with nc.allow_low_precision("int-valued degree sums <2048 exact in f16"):
                nc.vector.reduce_sum(rd16[:], dat_sb[:], axis=mybir.AxisListType.X)
            rd = cpool.tile([128, NWJ], F32)
            nc.vector.tensor_copy(rd[:], rd16[:])
            t1 = cpool.tile([128, NWJ], F32)
            nc.vector.tensor_scalar(
                out=t1[:], in0=rd[:], scalar1=1.0, scalar2=None,
                op0=mybir.AluOpType.max,
            )
            nc.scalar.sqrt(t1[:], t1[:])
            rcp = cpool.tile([128, NWJ], F32)
            nc.vector.reciprocal(rcp[:], t1[:])
            fmask = cpool.tile([128, NWJ], F32)
            nc.vector.tensor_scalar(
                out=fmask[:], in0=rd[:], scalar1=0.5, scalar2=None,
                op0=mybir.AluOpType.is_gt,
            )
            f_sb = cpool.tile([128, NWJ], F32)
            nc.vector.tensor_tensor(
                out=f_sb[:], in0=rcp[:], in1=fmask[:], op=mybir.AluOpType.mult
            )
            finv16 = cpool.tile([128, NWJ], F16)
            nc.vector.tensor_tensor(
                out=finv16[:], in0=t1[:], in1=fmask[:], op=mybir.AluOpType.mult
            )

            # --- A blocks: one local_scatter per (quarter, window) ----
            a_sb = {}
            for q in range(NQ):
                for j in range(NWJ):
                    a_t = apool.tile([128, QW], F16, tag="A", name=f"a{q}_{j}")
                    base = j * (NQ * KMAX) + q * KMAX
                    nc.gpsimd.local_scatter(
                        out_ap=a_t[:],
                        data_ap=dat_sb[:, j, q * KMAX:(q + 1) * KMAX],
                        idxs_ap=idx_sb[:, base:base + KMAX],
                        channels=128,
                        num_elems=QW,
                        num_idxs=KMAX,
                    )
                    a_sb[(q, j)] = a_t

            # --- xw = x_shard @ W (f32r), cast to f16 -----------------
            xw_sb = []
            for j in range(NWJ):
                pxw = ppool.tile([128, OC], F32, tag="xwp", bufs=2, name=f"pxw{j}")
                for k in range(IC // 128):
                    nc.tensor.matmul(
                        pxw[:],
                        x_sb[:, k, j * 128:(j + 1) * 128],
                        w_sb[:, k, :],
                        start=(k == 0),
                        stop=(k == IC // 128 - 1),
                    )
                xw_t = cpool.tile([128, OC], F16, tag="XW", bufs=NWJ,
                                  name=f"xw{j}")
                nc.scalar.copy(xw_t[:], pxw[:])
                xw_sb.append(xw_t)

            # --- scale A rows by f in place (DVE/Act alternating) -----
            for q in range(NQ):
                for j in range(NWJ):
                    if j % 2 == 0:
                        nc.vector.tensor_scalar(
                            out=a_sb[(q, j)][:], in0=a_sb[(q, j)][:],
                            scalar1=f_sb[:, j:j + 1], scalar2=None,
                            op0=mybir.AluOpType.mult,
                        )
                    else:
                        nc.scalar.mul(
                            a_sb[(q, j)][:], a_sb[(q, j)][:], f_sb[:, j:j + 1]
                        )

            # --- partial GEMM + col_deg payload, flush to rsin --------
            for q in range(NQ):
                for t in range(QW // 128):
                    pt = q * (QW // 128) + t
                    pp = ppool.tile([128, OC], F32, tag="pp", bufs=2,
                                    name=f"pp{pt}")
                    pcd = ppool.tile([128, OC], F32, tag="cd", bufs=2,
                                     name=f"pcd{pt}")
                    for j in range(NWJ):
                        nc.tensor.matmul(
                            pp[:],
                            a_sb[(q, j)][:, t * 128:(t + 1) * 128],
                            xw_sb[j][:],
                            start=(j == 0),
                            stop=(j == NWJ - 1),
                        )
                        nc.tensor.matmul(
                            pcd[:, 0:1],
                            a_sb[(q, j)][:, t * 128:(t + 1) * 128],
                            finv16[:, j:j + 1],
                            start=(j == 0),
                            stop=(j == NWJ - 1),
                        )
                    st = wpool.tile([128, PC], F16, tag="st", bufs=3,
                                    name=f"st{pt}")
                    if pt % 2 == 0:
                        nc.scalar.copy(st[:, 0:OC], pp[:])
                    else:
                        nc.vector.tensor_copy(st[:, 0:OC], pp[:])
                    nc.vector.tensor_copy(st[:, OC:OC + 1], pcd[:, 0:1])
                    nc.vector.memset(st[:, OC + 1:PC], 0.0)
                    nc.sync.dma_start(rsin[pt * 128:(pt + 1) * 128, :], st[:])

            # --- ReduceScatter: sum partials, keep my dst window ------
            nc.gpsimd.collective_compute(
                "ReduceScatter",
                mybir.AluOpType.add,
                replica_groups=[list(range(CORES))],
                ins=[rsin[:].opt()],
                outs=[rsout[:].opt()],
            )

            # --- finalize: g scale + bias -----------------------------
            rs_sb = cpool.tile([128, NWJ, PC], F16)
            for v in range(NWJ):
                nc.sync.dma_start(rs_sb[:, v, :], rsout[v])
            cdc = cpool.tile([128, NWJ], F32)
            nc.vector.tensor_copy(cdc[:], rs_sb[:, :, OC])
            g1 = cpool.tile([128, NWJ], F32)
            nc.vector.tensor_scalar(
                out=g1[:], in0=cdc[:], scalar1=1.0, scalar2=None,
                op0=mybir.AluOpType.max,
            )
            nc.scalar.sqrt(g1[:], g1[:])
            grcp = cpool.tile([128, NWJ], F32)
            nc.vector.reciprocal(grcp[:], g1[:])
            gmask = cpool.tile([128, NWJ], F32)
            nc.vector.tensor_scalar(
                out=gmask[:], in0=cdc[:], scalar1=0.5, scalar2=None,
                op0=mybir.AluOpType.is_gt,
            )
            g_sb = cpool.tile([128, NWJ], F32)
            nc.vector.tensor_tensor(
                out=g_sb[:], in0=grcp[:], in1=gmask[:], op=mybir.AluOpType.mult
            )
            for v in range(NWJ):
                og = wpool.tile([128, OC], F32, tag="og", name=f"og{v}")
                nc.scalar.mul(og[:], rs_sb[:, v, 0:OC], g_sb[:, v:v + 1])
                nc.vector.tensor_tensor(
                    out=og[:], in0=og[:], in1=brep_sb[:], op=mybir.AluOpType.add
                )
                nc.sync.dma_start(out[v * 128:(v + 1) * 128, :], og[:])

    nc.finalize()
    return nc


def make_in_maps(x, weight, bias, edge_index):
    """Host-side sharding/layout only: grouping, dedup and padding of the
    edge list (index preprocessing); no arithmetic on float tensor data."""
    x = np.asarray(x, dtype=np.float32)
    weight = np.ascontiguousarray(np.asarray(weight, dtype=np.float32))
    bias = np.asarray(bias, dtype=np.float32)
    ei = np.asarray(edge_index)
    s_all = ei[0].astype(np.int64)
    d_all = ei[1].astype(np.int64)
    assert s_all.min() >= 0 and s_all.max() < ND, "src ids out of supported range"
    assert d_all.min() >= 0 and d_all.max() < ND, "dst ids out of supported range"

    brep = np.ascontiguousarray(np.tile(bias[None, :], (128, 1)).astype(np.float32))
    whr = np.ascontiguousarray(weight.reshape(IC // 128, 128, OC).transpose(1, 0, 2))

    core_of = s_all >> 9
    in_maps = []
    for c in range(CORES):
        m = core_of == c
        gl = s_all[m] - c * GSH           # local gene id [0, 512)
        d = d_all[m]                      # full dst id [0, 4000)
        key = gl * NDP + d
        uniq, cnt = np.unique(key, return_counts=True)
        gu = uniq // NDP
        du = uniq % NDP
        j = gu >> 7
        p = gu & 127
        q = du >> 10
        dloc = du & (QW - 1)
        # slot rank within each (j, q, p) group (uniq sorted => groups together)
        gk = (gu << 2) | q
        _, start_idx = np.unique(gk, return_index=True)
        starts = np.zeros(len(gk), dtype=np.int64)
        starts[start_idx] = start_idx
        starts = np.maximum.accumulate(starts)
        rank = np.arange(len(gk)) - starts
        assert rank.max() < KMAX, f"slot overflow: {rank.max() + 1} > {KMAX}"

        idx_t = np.full((128, NWJ * NQ * KMAX), -1, dtype=np.int16)
        dat_t = np.zeros((128, NWJ * NQ * KMAX), dtype=np.float16)
        col = j * (NQ * KMAX) + q * KMAX + rank
        idx_t[p, col] = dloc.astype(np.int16)
        dat_t[p, col] = cnt.astype(np.float16)

        xs = np.zeros((GSH, IC), dtype=np.float32)
        n = min(GSH, ND - c * GSH)
        xs[:n] = x[c * GSH:c * GSH + n]
        xhr = np.ascontiguousarray(xs.T.reshape(IC // 128, 128, GSH).transpose(1, 0, 2))

        in_maps.append(
            {
                "xh": xhr,
                "wh": whr,
                "brep": brep,
                "idxt": idx_t,
                "datt": dat_t.reshape(128, NWJ, NQ * KMAX),
            }
        )
    return in_maps


_NC = None


def _get_nc():
    global _NC
    if _NC is None:
        _NC = build_nc()
    return _NC


def kernel(x, weight, bias, edge_index, **run_kwargs):
    from concourse.bass_utils import run_bass_kernel_spmd

    nc = _get_nc()
    in_maps = make_in_maps(x, weight, bias, edge_index)
    res = run_bass_kernel_spmd(nc, in_maps, core_ids=list(range(CORES)), **run_kwargs)
    outs = res.results if hasattr(res, "results") else res
    full = np.empty((ND, OC), dtype=np.float32)
    for c in range(CORES):
        n = min(GSH, ND - c * GSH)
        full[c * GSH:c * GSH + n] = outs[c]["out"][:n]
    if run_kwargs:
        return full, res
    return full


# revision 8
# speedup vs baseline: 4.6439x; 1.0711x over previous
"""BipartiteGCN message-passing kernel for 8 TRN2 NeuronCores.

Math:  out = D_c^{-1/2} A^T D_r^{-1/2} (x @ W) + b
where A[s, d] = multiplicity of edge (gene s, drug d), s, d in [0, 4000).

Strategy (gene sharding + one f16 ReduceScatter):
  - Core c owns gene range [512c, 512c+512) and ALL drug columns.  The
    host groups that core's edges by (gene row, 128-gene window, dst
    quarter), dedupes per (gene,dst) pair into (dst-index, multiplicity)
    slots (pure index layout, no float arithmetic).
  - Each [128 x 1024] block of the dense count stripe A (f16) is built
    in SBUF by ONE gpsimd local_scatter (per-partition indices,
    multiplicity payload).  No one-hot matmuls, no gather/scatter DMA.
  - row_deg is fully local (a core owns complete gene rows): one
    f16 reduce over the multiplicity table.  f = rsqrt-masked row_deg
    scales A in place.  No AllReduce.
  - col_deg rides along as payload column 512 of the partial GEMM
    output: tiny rank-1 matmuls (f*A)^T @ (1/f) accumulate per-dst
    column sums in PSUM.
  - xw = x_shard @ W in f32r (1 cycle/row), cast to f16.
  - partials[d, :512] = (f*A)^T @ xw over the 512 local genes; one
    ReduceScatter(add) over the [4096, 520] f16 partials both sums the
    8 per-core partials and hands core c exactly its 512-drug window.
  - out = g * partial + bias ; host concatenates the dst stripes.
"""

import sys

if "/opt/trn_rl_repo" not in sys.path:
    sys.path.insert(0, "/opt/trn_rl_repo")

import numpy as np

import concourse.bass as bass  # noqa: F401
import concourse.mybir as mybir
from concourse import bacc, tile

CORES = 8
GSH = 512               # genes per core
NWJ = GSH // 128        # 4 local gene windows
ND = 4000               # number of drugs
NDP = 4096              # padded drug dim
NQ = 4                  # dst quarters
QW = NDP // NQ          # 1024 dst per quarter
IC = 1024
OC = 512
PC = 514                # RS payload columns: 512 out + col_deg + pad
KMAX = 48               # max deduped slots per (gene row, window, quarter)

F32 = mybir.dt.float32
F32R = mybir.dt.float32r
F16 = mybir.dt.float16
I16 = mybir.dt.int16


def build_nc(debug_outputs=False):
    nc = bacc.Bacc(
        None,
        target_bir_lowering=False,
        debug=False,
        num_devices=CORES,
    )

    # xh[p, k, gl] = x[512c + gl, 128k + p]  (lhsT layout for x @ W)
    xh = nc.dram_tensor("xh", [128, IC // 128, GSH], F32R, kind="ExternalInput")
    wh = nc.dram_tensor("wh", [128, IC // 128, OC], F32R, kind="ExternalInput")
    brep = nc.dram_tensor("brep", [128, OC], F32, kind="ExternalInput")
    # slot tables: col = j*(NQ*KMAX) + q*KMAX + k
    idxt = nc.dram_tensor("idxt", [128, NWJ * NQ * KMAX], I16, kind="ExternalInput")
    datt = nc.dram_tensor("datt", [128, NWJ, NQ * KMAX], F16, kind="ExternalInput")
    out = nc.dram_tensor("out", [GSH, OC], F32, kind="ExternalOutput")

    rsin = nc.dram_tensor("rsin", [NDP, PC], F16)
    rsout = nc.dram_tensor("rsout", [NWJ, 128, PC], F16)

    with tile.TileContext(nc) as tc:
        with (
            tc.tile_pool(name="const", bufs=1) as cpool,
            tc.tile_pool(name="work", bufs=2) as wpool,
            tc.tile_pool(name="apool", bufs=NWJ * NQ) as apool,
            tc.tile_pool(name="psum", bufs=4, space="PSUM") as ppool,
        ):
            # --- input loads (small index tables first, then w, then x
            # in per-window blocks so xw matmuls can start early) -------
            idx_sb = cpool.tile([128, NWJ * NQ * KMAX], I16)
            nc.sync.dma_start(idx_sb[:], idxt[:])
            dat_sb = cpool.tile([128, NWJ, NQ * KMAX], F16)
            nc.sync.dma_start(dat_sb[:], datt[:])
            w_sb = cpool.tile([128, IC // 128, OC], F32R)
            nc.sync.dma_start(w_sb[:], wh[:])
            x_sb = cpool.tile([128, IC // 128, GSH], F32R)
            for j in range(NWJ):
                nc.sync.dma_start(
                    x_sb[:, :, j * 128:(j + 1) * 128],
                    xh[:, :, j * 128:(j + 1) * 128],
                )
            brep_sb = cpool.tile([128, OC], F32)
            nc.sync.dma_start(brep_sb[:], brep[:])

            # --- row_deg (local!) + f, 1/f ----------------------------
            rd16 = cpool.tile([128, NWJ], F16)
            with nc.allow_low_precision("int-valued degree sums <2048 exact in f16"):
                nc.vector.reduce_sum(rd16[:], dat_sb[:], axis=mybir.AxisListType.X)
            # rows with deg 0 have all-zero A rows, so no rsqrt masking is
            # needed: f and 1/f multiply zeros.
            t1 = cpool.tile([128, NWJ], F32)
            nc.vector.tensor_scalar(
                out=t1[:], in0=rd16[:], scalar1=1.0, scalar2=None,
                op0=mybir.AluOpType.max,
            )
            nc.scalar.sqrt(t1[:], t1[:])
            f_sb = cpool.tile([128, NWJ], F32)
            nc.vector.reciprocal(f_sb[:], t1[:])
            finv16 = cpool.tile([128, NWJ], F16)
            nc.vector.tensor_copy(finv16[:], t1[:])

            # --- A blocks: one local_scatter per (quarter, window) ----
            a_sb = {}
            for q in range(NQ):
                for j in range(NWJ):
                    a_t = apool.tile([128, QW], F16, tag="A", name=f"a{q}_{j}")
                    base = j * (NQ * KMAX) + q * KMAX
                    nc.gpsimd.local_scatter(
                        out_ap=a_t[:],
                        data_ap=dat_sb[:, j, q * KMAX:(q + 1) * KMAX],
                        idxs_ap=idx_sb[:, base:base + KMAX],
                        channels=128,
                        num_elems=QW,
                        num_idxs=KMAX,
                    )
                    a_sb[(q, j)] = a_t

            # --- xw = x_shard @ W (f32r), cast to f16 -----------------
            xw_sb = []
            for j in range(NWJ):
                pxw = ppool.tile([128, OC], F32, tag="xwp", bufs=2, name=f"pxw{j}")
                for k in range(IC // 128):
                    nc.tensor.matmul(
                        pxw[:],
                        x_sb[:, k, j * 128:(j + 1) * 128],
                        w_sb[:, k, :],
                        start=(k == 0),
                        stop=(k == IC // 128 - 1),
                    )
                xw_t = cpool.tile([128, OC], F16, tag="XW", bufs=NWJ,
                                  name=f"xw{j}")
                nc.scalar.copy(xw_t[:], pxw[:])
                xw_sb.append(xw_t)

            # --- scale A rows by f in place (DVE/Act alternating) -----
            for q in range(NQ):
                for j in range(NWJ):
                    if j % 2 == 0:
                        nc.vector.tensor_scalar(
                            out=a_sb[(q, j)][:], in0=a_sb[(q, j)][:],
                            scalar1=f_sb[:, j:j + 1], scalar2=None,
                            op0=mybir.AluOpType.mult,
                        )
                    else:
                        nc.scalar.mul(
                            a_sb[(q, j)][:], a_sb[(q, j)][:], f_sb[:, j:j + 1]
                        )

            # --- partial GEMM + col_deg payload, flush to rsin --------
            for q in range(NQ):
                for t in range(QW // 128):
                    pt = q * (QW // 128) + t
                    pp = ppool.tile([128, OC], F32, tag="pp", bufs=3,
                                    name=f"pp{pt}")
                    pcd = ppool.tile([128, OC], F32, tag="cd", bufs=2,
                                     name=f"pcd{pt}")
                    for j in range(NWJ):
                        nc.tensor.matmul(
                            pp[:],
                            a_sb[(q, j)][:, t * 128:(t + 1) * 128],
                            xw_sb[j][:],
                            start=(j == 0),
                            stop=(j == NWJ - 1),
                        )
                        nc.tensor.matmul(
                            pcd[:, 0:1],
                            a_sb[(q, j)][:, t * 128:(t + 1) * 128],
                            finv16[:, j:j + 1],
                            start=(j == 0),
                            stop=(j == NWJ - 1),
                        )
                    st = wpool.tile([128, PC], F16, tag="st", bufs=3,
                                    name=f"st{pt}")
                    if pt % 2 == 0:
                        nc.scalar.copy(st[:, 0:OC], pp[:])
                    else:
                        nc.vector.tensor_copy(st[:, 0:OC], pp[:])
                    nc.vector.tensor_copy(st[:, OC:OC + 1], pcd[:, 0:1])
                    nc.vector.memset(st[:, OC + 1:PC], 0.0)
                    nc.sync.dma_start(rsin[pt * 128:(pt + 1) * 128, :], st[:])

            # --- ReduceScatter: sum partials, keep my dst window ------
            nc.gpsimd.collective_compute(
                "ReduceScatter",
                mybir.AluOpType.add,
                replica_groups=[list(range(CORES))],
                ins=[rsin[:].opt()],
                outs=[rsout[:].opt()],
            )

            # --- finalize: g scale + bias -----------------------------
            rs_sb = cpool.tile([128, NWJ, PC], F16)
            for v in range(NWJ):
                nc.sync.dma_start(rs_sb[:, v, :], rsout[v])
            # empty dst columns have zero partials, so g needs no mask.
            g1 = cpool.tile([128, NWJ], F32)
            nc.vector.tensor_scalar(
                out=g1[:], in0=rs_sb[:, :, OC], scalar1=1.0, scalar2=None,
                op0=mybir.AluOpType.max,
            )
            nc.scalar.sqrt(g1[:], g1[:])
            g_sb = cpool.tile([128, NWJ], F32)
            nc.vector.reciprocal(g_sb[:], g1[:])
            for v in range(NWJ):
                og = wpool.tile([128, OC], F32, tag="og", name=f"og{v}")
                nc.scalar.mul(og[:], rs_sb[:, v, 0:OC], g_sb[:, v:v + 1])
                nc.vector.tensor_tensor(
                    out=og[:], in0=og[:], in1=brep_sb[:], op=mybir.AluOpType.add
                )
                nc.sync.dma_start(out[v * 128:(v + 1) * 128, :], og[:])

    nc.finalize()
    return nc


def make_in_maps(x, weight, bias, edge_index):
    """Host-side sharding/layout only: grouping, dedup and padding of the
    edge list (index preprocessing); no arithmetic on float tensor data."""
    x = np.asarray(x, dtype=np.float32)
    weight = np.ascontiguousarray(np.asarray(weight, dtype=np.float32))
    bias = np.asarray(bias, dtype=np.float32)
    ei = np.asarray(edge_index)
    s_all = ei[0].astype(np.int64)
    d_all = ei[1].astype(np.int64)
    assert s_all.min() >= 0 and s_all.max() < ND, "src ids out of supported range"
    assert d_all.min() >= 0 and d_all.max() < ND, "dst ids out of supported range"

    brep = np.ascontiguousarray(np.tile(bias[None, :], (128, 1)).astype(np.float32))
    whr = np.ascontiguousarray(weight.reshape(IC // 128, 128, OC).transpose(1, 0, 2))

    core_of = s_all >> 9
    in_maps = []
    for c in range(CORES):
        m = core_of == c
        gl = s_all[m] - c * GSH           # local gene id [0, 512)
        d = d_all[m]                      # full dst id [0, 4000)
        key = gl * NDP + d
        uniq, cnt = np.unique(key, return_counts=True)
        gu = uniq // NDP
        du = uniq % NDP
        j = gu >> 7
        p = gu & 127
        q = du >> 10
        dloc = du & (QW - 1)
        # slot rank within each (j, q, p) group (uniq sorted => groups together)
        gk = (gu << 2) | q
        _, start_idx = np.unique(gk, return_index=True)
        starts = np.zeros(len(gk), dtype=np.int64)
        starts[start_idx] = start_idx
        starts = np.maximum.accumulate(starts)
        rank = np.arange(len(gk)) - starts
        assert rank.max() < KMAX, f"slot overflow: {rank.max() + 1} > {KMAX}"

        idx_t = np.full((128, NWJ * NQ * KMAX), -1, dtype=np.int16)
        dat_t = np.zeros((128, NWJ * NQ * KMAX), dtype=np.float16)
        col = j * (NQ * KMAX) + q * KMAX + rank
        idx_t[p, col] = dloc.astype(np.int16)
        dat_t[p, col] = cnt.astype(np.float16)

        xs = np.zeros((GSH, IC), dtype=np.float32)
        n = min(GSH, ND - c * GSH)
        xs[:n] = x[c * GSH:c * GSH + n]
        xhr = np.ascontiguousarray(xs.T.reshape(IC // 128, 128, GSH).transpose(1, 0, 2))

        in_maps.append(
            {
                "xh": xhr,
                "wh": whr,
                "brep": brep,
                "idxt": idx_t,
                "datt": dat_t.reshape(128, NWJ, NQ * KMAX),
            }
        )
    return in_maps


_NC = None


def _get_nc():
    global _NC
    if _NC is None:
        _NC = build_nc()
    return _NC


def kernel(x, weight, bias, edge_index, **run_kwargs):
    from concourse.bass_utils import run_bass_kernel_spmd

    nc = _get_nc()
    in_maps = make_in_maps(x, weight, bias, edge_index)
    res = run_bass_kernel_spmd(nc, in_maps, core_ids=list(range(CORES)), **run_kwargs)
    outs = res.results if hasattr(res, "results") else res
    full = np.empty((ND, OC), dtype=np.float32)
    for c in range(CORES):
        n = min(GSH, ND - c * GSH)
        full[c * GSH:c * GSH + n] = outs[c]["out"][:n]
    if run_kwargs:
        return full, res
    return full
